# revision 35
# baseline (speedup 1.0000x reference)
"""DPQ (product-quantization) network kernel for Trainium2, 8 NeuronCores.

Problem: inputs [32768, 32, 16], centroids [32, 256, 16], W [512, 512].
  response[b,c,k] = <inputs[b,c,:], centroids[c,k,:]>
  neg_mse = max_k response;  codes = argmax_k response
  outputs[b,c] = centroids[c, codes[b,c]];  product = outputs.reshape(B,512) @ W
Returns (product, -neg_mse, codes).

Strategy (data-parallel over batch, 4096 rows/core):
  * host: transpose inputs to [512, B]; build block-diagonal centroid pair
    matrices so each PE matmul computes two subspaces' responses with K=32
    contraction, concurrent via tile_position row strips (exact fp32;
    the final FC runs in fp32r at 1 cycle/row).
  * argmax: ONE custom DVE pass over the response in PSUM. The custom op
    (patched segmented scan, resets at 8-element page boundaries) packs
    quantized value + within-page index into one fp32:
        w = rne(r * 2^18 to multiple of 8) + (7 - widx)
    Page winners land at out[:, s, 7]. Two tiny segmented passes then find
    the best page (exact compare + first-match position) and small ACT/DVE
    ops decode codes and -neg_mse (max quantized to 2^-15 absolute).
  * gather: gpsimd indirect_copy produces outputsT [(c,d), b] directly from
    a per-partition centroid-column table; index layout built by a small
    SBUF->SBUF shuffle DMA. FC = 4 accumulating matmuls against W.
"""

import os
import sys
import time

sys.path.insert(0, "/opt/trn_rl_repo")

import numpy as np

try:
    import jax

    jax.config.update("jax_compilation_cache_dir", "/tmp/dpq_jax_cache")
    jax.config.update("jax_persistent_cache_min_compile_time_secs", 1.0)
except Exception:
    pass

import concourse.bass as bass
import concourse.tile as tile
from concourse import bacc, bass_utils, mybir
from concourse import dve_spec as ds
from concourse import dve_ops as dops
from concourse.bass_interp import get_hw_module
from concourse.dve_uop import AluOp, DveOpSpec, N_LANES, N_STAGES, Trigger

F32 = np.float32
N_CORES = 8
B, C, K, D = 32768, 32, 256, 16
B_CORE = B // N_CORES          # 4096
NBT = B_CORE // 128            # 32 b-tiles per core
PAGE = 8                       # level-1 page size (index bits: 3)
SCALE = float(2 ** 18)         # response pre-scale
MQ = float(2 ** 26)            # quantize-to-8 offset (ulp(2^26)=8)
GRID = 2.0 ** -15              # resulting absolute quantization of max
RNE = float(2 ** 23)           # round-to-nearest-integer offset

# ---------------------------------------------------------------------------
# Custom DVE ops: segmented scans with SUB_DIM_DONE reset (hand-lowered).
# ---------------------------------------------------------------------------


def _lower_segmented(spec, ver):
    """lower() clone adding a SUB_DIM_DONE step state: SUBTRACT-scans reset to
    C1 (descending per-page counter), MAX-scans reset to their own body input
    (running max restarts each page). ADD-scans (global Idx) do not reset."""
    n_lanes, n_stages = N_LANES[ver], N_STAGES[ver]
    ds._validate_body(spec, ver)
    spec2 = ds._hoist_stream_invariant_ops(spec)
    scans = ds._collect(spec2.body, ds.Scan)
    latches = ds._collect(spec2.body, ds.Latch)
    placement = ds._build_placement(spec2, scans, n_stages, n_lanes)
    states = ds._build_state_machine(spec2, scans, latches, placement)

    step_ov = {}
    for scan in scans:
        d = placement.node_stage[scan]
        if scan._subdim_step is not None:
            step_ov[d] = ds._Stage(scan.op, ds.AluInp.CURR_ALU_OUT, scan._subdim_step)
        elif scan.op == AluOp.SUBTRACT:
            step_ov[d] = ds._Stage(AluOp.BYPASS, ds.C1)
        elif scan.op == AluOp.MAX:
            step_ov[d] = ds._Stage(AluOp.BYPASS, scan.expr)
        elif scan.op == AluOp.ADD:
            pass
        else:
            raise ValueError(f"unsupported segmented scan op {scan.op}")

    steady = states[-1]
    assert steady.trigger == ds.SRC_DONE and not steady.overrides
    steady_idx = len(states) - 1
    step_idx = steady_idx + 1
    states = states[:-1]
    states.append(ds._State(
        placement=placement, consume=steady.consume,
        trigger=(Trigger.SRC_TENSOR_DONE, Trigger.SUB_DIM_DONE, Trigger.NONE),
        next=(0, step_idx, 0)))
    states.append(ds._State(
        placement=placement, consume=steady.consume, overrides=step_ov,
        trigger=(Trigger.SRC_TENSOR_DONE, Trigger.SUB_DIM_DONE, Trigger.COUNT),
        next=(0, step_idx, steady_idx), repeat=1))
    uops = [ds._assemble(s) for s in states]
    for u in uops:
        u.validate(ver)
    return uops, ds._has_src1(spec2)


def _raw_scan(op, expr, init):
    """Scan node bypassing the nested-scan API guard (HW-validated)."""
    s = ds.Scan.__new__(ds.Scan)
    object.__setattr__(s, "op", op)
    object.__setattr__(s, "expr", expr)
    object.__setattr__(s, "init", init)
    object.__setattr__(s, "_subdim_step", None)
    return s


def _ref_segpack(in0, in1, c0, c1, c2):
    x = np.asarray(in0, F32)
    n = x.shape[2]
    t = x * F32(c2)
    u = (t + F32(c0)).astype(F32)
    v = (u - F32(c0)).astype(F32)
    rev = (F32(c1) - np.arange(n, dtype=F32)).astype(F32)
    return np.maximum.accumulate((v + rev).astype(F32), axis=2)


def _ref_segmax(in0, in1, c0, c1, c2):
    return np.maximum.accumulate(np.asarray(in0, F32), axis=2)


def _ref_segpos(in0, in1, c0, c1, c2):
    x = np.asarray(in0, F32)
    y = np.asarray(in1, F32)
    s, n = x.shape[1], x.shape[2]
    idx = np.arange(s * n, dtype=F32).reshape(1, s, n)
    cand = np.where(x == y, F32(c1) - idx, F32(-3.4028235e38)).astype(F32)
    return np.maximum.accumulate(cand, axis=2)


_REGISTERED = {}


def _register(name, spec, ver):
    if name in _REGISTERED:
        return _REGISTERED[name]
    uops, rd1 = _lower_segmented(spec, ver)
    row = max(dops._SUB_OPCODE_FOR_NAME.values()) + 1
    assert row < 0x20
    op = dops.DveOp(name, spec, subdim=True, uops_sha={})
    dops.OPS.append(op)
    dops.CUSTOM_DVE_SPECS[name] = spec
    dops._SUB_OPCODE_FOR_NAME[name] = row
    dops._COMPILE_CACHE[(name, ver)] = DveOpSpec(
        name=name, opcode=row, uops=uops, rd1_en=rd1)
    _REGISTERED[name] = op
    return op


def _get_ops(ver="v3"):
    dsc = ds.scan(AluOp.SUBTRACT, ds.One, init=ds.C1 + ds.One)
    t = ds.Src0 * ds.C2
    u = t + ds.C0
    v = u - ds.C0
    w = v + dsc
    segpack = ds.Spec(body=_raw_scan(AluOp.MAX, w, ds.MaxNeg),
                      reference=_ref_segpack)
    segmax = ds.Spec(body=ds.scan(AluOp.MAX, ds.Src0, init=ds.MaxNeg),
                     reference=_ref_segmax)
    cand = ds.select(ds.eq(ds.Src0, ds.Src1), ds.C1 - ds.Idx, ds.MaxNeg)
    segpos = ds.Spec(body=_raw_scan(AluOp.MAX, cand, ds.MaxNeg),
                     reference=_ref_segpos)
    return (_register("DPQ_SEGPACK", segpack, ver),
            _register("DPQ_SEGMAX", segmax, ver),
            _register("DPQ_SEGPOS", segpos, ver))


# ---------------------------------------------------------------------------
# Device program (one core; SPMD across 8).
# ---------------------------------------------------------------------------

_COMPILED = None


def _build():
    segpack, segmax, segpos = _get_ops("v3")
    f32, f32r = mybir.dt.float32, mybir.dt.float32r
    i32, u16 = mybir.dt.int32, mybir.dt.uint16

    nc = bacc.Bacc("TRN2", target_bir_lowering=False, debug=False,
                   num_devices=N_CORES)
    xT = nc.dram_tensor("xT", [512, B_CORE], f32, kind="ExternalInput")
    cpair = nc.dram_tensor("cpair", [512, 512], f32, kind="ExternalInput")
    centT = nc.dram_tensor("centT", [512, 256], f32, kind="ExternalInput")
    Wt = nc.dram_tensor("W", [512, 512], f32, kind="ExternalInput")
    ccol = nc.dram_tensor("ccol", [128, 32], f32, kind="ExternalInput")
    ident = nc.dram_tensor("ident", [128, 128], f32, kind="ExternalInput")
    prod = nc.dram_tensor("prod", [B_CORE, 512], f32, kind="ExternalOutput")
    negmse = nc.dram_tensor("negmse", [B_CORE, 32], f32, kind="ExternalOutput")
    codes = nc.dram_tensor("codes", [B_CORE, 32], i32, kind="ExternalOutput")

    CHUNK = 1024                      # batch rows staged per xT load
    NCH = B_CORE // CHUNK             # 4 chunks
    BT_PER = CHUNK // 128             # 8 b-tiles per chunk

    with tile.TileContext(nc) as tc:
        with (
            tc.tile_pool(name="const", bufs=1) as cpool,
            tc.tile_pool(name="x", bufs=2) as xpool,
            tc.tile_pool(name="resp", bufs=3, space="PSUM") as qpool,
            tc.tile_pool(name="fcp", bufs=1, space="PSUM") as fcpool,
            tc.tile_pool(name="tpp", bufs=1, space="PSUM") as tppool,
            tc.tile_pool(name="l1", bufs=3) as l1pool,
            tc.tile_pool(name="lvl2", bufs=3) as l2pool,
            tc.tile_pool(name="dec", bufs=3) as dpool,
            tc.tile_pool(name="gth", bufs=3) as gpool,
            tc.tile_pool(name="out", bufs=2) as opool,
        ):
            # constants (matmul operands converted once to fp32r: the PE
            # streams fp32r at 1 cycle/row vs 4 for fp32)
            cp_sb = []
            ct_sb = []
            w_sb = []
            for g in range(4):
                t = cpool.tile([128, 512], f32, tag=f"cp{g}")
                nc.sync.dma_start(t[:], cpair[128 * g:128 * (g + 1), :])
                cp_sb.append(t)
                t = cpool.tile([128, 256], f32, tag=f"ct{g}")
                nc.sync.dma_start(t[:], centT[128 * g:128 * (g + 1), :])
                ct_sb.append(t)
                t = cpool.tile([128, 512], f32, tag=f"w{g}0")
                nc.sync.dma_start(t[:], Wt[128 * g:128 * (g + 1), :])
                tr = cpool.tile([128, 512], f32r, tag=f"w{g}")
                nc.scalar.copy(tr[:], t[:])
                w_sb.append(tr)
            cc_sb = cpool.tile([128, 32], f32, tag="ccol")
            nc.sync.dma_start(cc_sb[:], ccol[:])
            id_sb = cpool.tile([128, 128], f32, tag="identX")
            nc.sync.dma_start(id_sb[:], ident[:])

            def load_chunk(ch):
                # first 128-column block in its own tiles so the chunk's
                # first b-tile starts after 256KB instead of 4MB of DMA
                xf = []
                xt = []
                for g in range(4):
                    t = xpool.tile([128, 128], f32, tag=f"xf{g}")
                    nc.sync.dma_start(
                        t[:], xT[128 * g:128 * (g + 1),
                                 CHUNK * ch:CHUNK * ch + 128])
                    xf.append(t)
                for g in range(4):
                    t = xpool.tile([128, CHUNK - 128], f32, tag=f"x{g}")
                    nc.sync.dma_start(
                        t[:], xT[128 * g:128 * (g + 1),
                                 CHUNK * ch + 128:CHUNK * (ch + 1)])
                    xt.append(t)
                return xf, xt

            def phase_a(bt_g, xfxt):
                    xf, xr = xfxt
                    b0 = (bt_g % BT_PER) * 128   # offset within chunk
                    xt = xf if b0 == 0 else xr
                    b0 = 0 if b0 == 0 else b0 - 128
                    # ---- response + level-1 packed segmented argmax ----
                    l1t = l1pool.tile([128, 8192], f32, tag="l1")
                    for h in range(8):
                        qt = qpool.tile([128, 1024], f32, tag="q")
                        for j2 in range(2):
                            p = 2 * h + j2
                            nc.tensor.matmul(
                                qt[:, 512 * j2:512 * (j2 + 1)],
                                lhsT=xt[p // 4][32 * (p % 4):32 * (p % 4) + 32,
                                               b0:b0 + 128],
                                rhs=cp_sb[p // 4][32 * (p % 4):32 * (p % 4) + 32, :],
                                start=True, stop=True,
                                tile_position=(32 * (p % 4), 0),
                            )
                        nc.vector._custom_dve(
                            segpack,
                            out=l1t[:, 1024 * h:1024 * (h + 1)].rearrange(
                                "p (s n) -> p s n", n=PAGE),
                            in0=qt[:].rearrange("p (s n) -> p s n", n=PAGE),
                            s0=MQ, s1=float(PAGE - 1), imm2=SCALE)
                    # ---- level 2: best page per (b, c) ----
                    ws = l1t[:].rearrange("p (c s n) -> p c s n", n=PAGE, s=K // PAGE)[:, :, :, PAGE - 1]
                    m2 = l2pool.tile([128, 1024], f32, tag="m2")
                    nc.vector._custom_dve(
                        segmax,
                        out=m2[:].rearrange("p (c s) -> p c s", s=K // PAGE),
                        in0=ws)
                    m2w = m2[:].rearrange("p (c s) -> p c s", s=K // PAGE)[:, :, K // PAGE - 1]
                    pos = l2pool.tile([128, 1024], f32, tag="pos")
                    nc.vector._custom_dve(
                        segpos,
                        out=pos[:].rearrange("p (c s) -> p c s", s=K // PAGE),
                        in0=ws,
                        in1=m2w.unsqueeze(2).broadcast_to([128, C, K // PAGE]),
                        s1=float(C * (K // PAGE) - 1))
                    w2b = pos[:].rearrange("p (c s) -> p c s", s=K // PAGE)[:, :, K // PAGE - 1]
                    # ---- decode ----
                    # n = round_ne(m2w/8 - 7/16)  (exact floor for rev in [0,7])
                    dn = dpool.tile([128, 32], f32, tag="dn")
                    nc.scalar.activation(dn[:], m2w,
                                         mybir.ActivationFunctionType.Copy,
                                         bias=-0.4375, scale=0.125)
                    nc.vector.tensor_scalar(dn[:], dn[:], RNE, -RNE,
                                            mybir.AluOpType.add,
                                            mybir.AluOpType.add)
                    ng = dpool.tile([128, 32], f32, tag="ng")
                    nc.scalar.mul(ng[:], dn[:], -GRID)
                    nc.sync.dma_start(negmse[128 * bt_g:128 * (bt_g + 1), :],
                                      ng[:])
                    # rev1 = m2w - 8n ; o4 = 8*w2b + rev1 ; k = ccol - o4
                    n8 = dpool.tile([128, 32], f32, tag="n8")
                    nc.scalar.mul(n8[:], dn[:], -8.0)
                    rv = dpool.tile([128, 32], f32, tag="rv")
                    nc.gpsimd.tensor_tensor(rv[:], m2w, n8[:],
                                            mybir.AluOpType.add)
                    w8 = dpool.tile([128, 32], f32, tag="w8")
                    nc.scalar.mul(w8[:], w2b, 8.0)
                    o4 = dpool.tile([128, 32], f32, tag="o4")
                    nc.gpsimd.tensor_tensor(o4[:], w8[:], rv[:],
                                            mybir.AluOpType.add)
                    kf = dpool.tile([128, 32], f32, tag="kf")
                    nc.gpsimd.tensor_tensor(kf[:], cc_sb[:], o4[:],
                                            mybir.AluOpType.subtract)
                    k32 = dpool.tile([128, 32], i32, tag="k32")
                    nc.scalar.copy(k32[:], kf[:])
                    nc.sync.dma_start(codes[128 * bt_g:128 * (bt_g + 1), :],
                                      k32[:])
                    return kf

            def phase_b1(bt_g, kf):
                    # codesT [32, 128] via PE transpose against a PERMUTATION
                    # matrix (not identity): kTI[c, q*8+s] = codes[16s+q, c],
                    # i.e. columns already in the gpsimd 16-partition-wrapped
                    # index order, so the idxs shuffle is a contiguous DMA.
                    tps = tppool.tile([32, 128], f32, tag="tp")
                    nc.tensor.transpose(tps[:], kf[:], id_sb[:])
                    kT16 = dpool.tile([32, 128], u16, tag="kT16")
                    nc.scalar.copy(kT16[:], tps[:])
                    # ---- gather (transposed) ----
                    otrs = []
                    for g in range(4):
                        idxs = gpool.tile([128, 8], u16, tag=f"ix{g}")
                        nc.sync.dma_start(idxs[:], kT16[8 * g:8 * (g + 1), :])
                        ot = gpool.tile([128, 128], f32, tag=f"ot{g}")
                        nc.gpsimd.indirect_copy(ot[:], ct_sb[g][:], idxs[:],
                                                i_know_ap_gather_is_preferred=True)
                        otr = gpool.tile([128, 128], f32r, tag=f"otr{g}")
                        nc.scalar.copy(otr[:], ot[:])
                        otrs.append(otr)
                    return otrs

            def phase_b2(bt_g, otrs):
                    fc = fcpool.tile([128, 512], f32, tag="fc")
                    for g in range(4):
                        nc.tensor.matmul(fc[:], lhsT=otrs[g][:],
                                         rhs=w_sb[g][:],
                                         start=(g == 0), stop=(g == 3))
                    ps = opool.tile([128, 512], f32, tag="ps")
                    nc.scalar.copy(ps[:], fc[:])
                    nc.sync.dma_start(prod[128 * bt_g:128 * (bt_g + 1), :],
                                      ps[:])

            # 2-deep software pipeline: b-tile i's response matmuls run while
            # i-1's transpose+gather chain completes off-PE and i-2's FC
            # consumes its already-gathered operands, so the PE never waits
            # on the ACT->DMA->gpsimd->ACT gather latency.
            xt = load_chunk(0)
            kfs = {}
            gathered = {}
            for bt_g in range(NBT):
                if bt_g and bt_g % BT_PER == 0:
                    xt = load_chunk(bt_g // BT_PER)
                kfs[bt_g] = phase_a(bt_g, xt)
                if bt_g >= 1:
                    gathered[bt_g - 1] = phase_b1(bt_g - 1, kfs.pop(bt_g - 1))
                if bt_g >= 2:
                    phase_b2(bt_g - 2, gathered.pop(bt_g - 2))
            gathered[NBT - 1] = phase_b1(NBT - 1, kfs.pop(NBT - 1))
            phase_b2(NBT - 2, gathered.pop(NBT - 2))
            phase_b2(NBT - 1, gathered.pop(NBT - 1))
    nc.compile()
    nc.m = get_hw_module(nc.m)
    return nc


def _get_compiled():
    global _COMPILED
    if _COMPILED is None:
        _COMPILED = _build()
    return _COMPILED


# ---------------------------------------------------------------------------
# Host side
# ---------------------------------------------------------------------------


def _host_prep(inputs, centroids, W):
    inputs = np.asarray(inputs, F32)
    centroids = np.asarray(centroids, F32)
    W = np.ascontiguousarray(np.asarray(W, F32))
    # xT[c*16+d, b]
    xT = np.ascontiguousarray(inputs.reshape(B, 512).T)
    # block-diag pair matrices: row p*32+r, pair p=(2p, 2p+1)
    cpair = np.zeros((16, 32, 512), F32)
    cT = centroids.transpose(0, 2, 1)          # [32, 16, 256]
    for p in range(16):
        cpair[p, 0:16, 0:256] = cT[2 * p]
        cpair[p, 16:32, 256:512] = cT[2 * p + 1]
    cpair = cpair.reshape(512, 512)
    # centT[(c%8)*16+d + 128*(c//8), k] = centroids[c, k, d]
    centT = np.ascontiguousarray(
        centroids.transpose(0, 2, 1).reshape(32 * 16, 256))
    ccol = np.broadcast_to(
        (8191.0 - 256.0 * np.arange(32, dtype=F32))[None, :], (128, 32)
    ).copy()
    # permutation for the codes transpose: column j of the transposed codes
    # holds batch row 16*(j%8) + j//8 (gpsimd 16-partition index wrap).
    ident = np.zeros((128, 128), F32)
    j = np.arange(128)
    ident[16 * (j % 8) + j // 8, j] = 1.0
    return xT, cpair, centT, W, ccol, ident


def kernel(inputs, centroids, W):
    xT, cpair, centT, Wm, ccol, ident = _host_prep(inputs, centroids, W)
    nc = _get_compiled()
    in_maps = []
    for c in range(N_CORES):
        in_maps.append({
            "xT": np.ascontiguousarray(
                xT[:, B_CORE * c:B_CORE * (c + 1)]),
            "cpair": cpair, "centT": centT, "W": Wm, "ccol": ccol,
            "ident": ident,
        })
    last_err = None
    for attempt in range(3):
        try:
            res = bass_utils.run_bass_kernel_spmd(
                nc, in_maps, list(range(N_CORES)),
                trace=bool(int(os.environ.get("DPQ_TRACE", "0"))))
            # materialize results inside the retry scope: jax dispatch is
            # async and transient device errors surface at fetch time
            prod = np.concatenate(
                [np.asarray(res.results[c]["prod"]) for c in range(N_CORES)], 0)
            negmse = np.concatenate(
                [np.asarray(res.results[c]["negmse"]) for c in range(N_CORES)], 0)
            codes = np.concatenate(
                [np.asarray(res.results[c]["codes"]) for c in range(N_CORES)], 0)
            break
        except Exception as e:  # transient NRT_EXEC_UNIT_UNRECOVERABLE seen
            last_err = e
            try:
                jax.clear_caches()
            except Exception:
                pass
            time.sleep(3.0 * (attempt + 1))
    else:
        raise last_err
    kernel._last_exec_time_ns = res.exec_time_ns
    kernel._last_results = res
    return prod, negmse, codes.astype(np.int32)
